# revision 9
# baseline (speedup 1.0000x reference)
"""Trainium2 Bass kernel for nn_Aggregate (2D rel-pos attention, 2 fmaps).

Math (per fmap, per batch, per head):
  q = SCALE * (Wq @ fmap)                      # (128, HW)  d x i, i=(x,y) H-major
  hs(x,y,u) = q(:,x,y) . rel_h[x-u+99]         # H-direction rel-pos logits
  ws(x,y,v) = q(:,x,y) . rel_w[y-v+99]         # W-direction rel-pos logits
  S(i, j=(u,v)) = hs + ws ; A = softmax_j(S)
  out = A @ V ; proj = gamma * Wp_h @ out

Key restructuring for TRN2:
  exp(hs+ws) = exp(hs) * exp(ws)  -- exp only on small factors (Eht, Ewt)
  softmax division deferred:  A@V = (E@V) / den,  den = (sum_u e^hs)(sum_v e^ws)
  E^T built chunk-by-chunk in (j-part, i-free) layout:
     E^T_c = EwtD  *  broadcast(Eht rows 2c, 2c+1)
  broadcast via DMA free-step-0 APs / gpsimd.partition_broadcast,
  multiply on DVE bf16 2x mode, attn@V on PE with K=128 chunks,
  denominators via ones-vector matmuls, division done on host (linearity).

Sharding: 16 head-instances = 2 fmaps x 2 batch x 4 heads -> 8 cores,
2 heads per core (same fmap/batch slice). Host sums the per-head
projection contributions and adds the residual.
"""
import numpy as np
import ml_dtypes
from contextlib import ExitStack

import concourse.bass as bass
import concourse.tile as tile
import concourse.mybir as mybir
from concourse import bacc, bass_utils
from concourse.bass_types import AP

F32 = mybir.dt.float32
BF16 = mybir.dt.bfloat16

HEADS = 4
DH = 128
DIM = 128
MAX_POS = 100
SCALE = DH ** -0.5
B = 2
H = 48
W = 64
HW = H * W          # 3072
NCHUNK = HW // 128  # 24
NBLK = HW // 512    # 6

# chunk build path per (head, chunk): 'dma' or 'gps'
GPS_CHUNKS = set()  # filled by tuning; e.g. {(0,0),(0,1),...}

_cached = {}


def _build_nc():
    if "nc" in _cached:
        return _cached["nc"]
    nc = bacc.Bacc("TRN2", target_bir_lowering=False, debug=False)

    fmapb_d = nc.dram_tensor("fmapb", [128, HW], BF16, kind="ExternalInput").ap()
    wqt_d = nc.dram_tensor("wqt", [128, 256], BF16, kind="ExternalInput").ap()
    wvt_d = nc.dram_tensor("wvt", [128, 256], BF16, kind="ExternalInput").ap()
    wpt_d = nc.dram_tensor("wpt", [128, 256], BF16, kind="ExternalInput").ap()
    het_d = nc.dram_tensor("het", [128, H * H], BF16, kind="ExternalInput").ap()
    wet_d = nc.dram_tensor("wet", [128, W * W], BF16, kind="ExternalInput").ap()
    out_d = [nc.dram_tensor(f"out{h}", [128, HW], F32, kind="ExternalOutput").ap()
             for h in range(2)]
    den_d = nc.dram_tensor("den", [4, HW], F32, kind="ExternalOutput").ap()

    with tile.TileContext(nc) as tc, ExitStack() as ctx:
        pool = ctx.enter_context(tc.tile_pool(name="sb", bufs=1))

        # ---- load inputs ----
        fmapb = pool.tile([128, HW], BF16)
        nc.sync.dma_start(fmapb[:], fmapb_d[:])
        wqt = pool.tile([128, 256], BF16)
        nc.sync.dma_start(wqt[:], wqt_d[:])
        wvt = pool.tile([128, 256], BF16)
        nc.sync.dma_start(wvt[:], wvt_d[:])
        wpt = pool.tile([128, 256], BF16)
        nc.sync.dma_start(wpt[:], wpt_d[:])
        het = pool.tile([128, H * H], BF16)
        nc.sync.dma_start(het[:], het_d[:])
        wet = pool.tile([128, W * W], BF16)
        nc.sync.dma_start(wet[:], wet_d[:])
        ones48 = pool.tile([48, 1], BF16)
        nc.vector.memset(ones48[:], 1.0)
        ones64 = pool.tile([64, 1], BF16)
        nc.vector.memset(ones64[:], 1.0)

        q2 = pool.tile([128, 2 * HW], BF16)      # (d, h*HW + i)
        v2 = pool.tile([128, NCHUNK * 256], BF16)  # (j_in_chunk, c*256 + h*128 + d)
        eht = pool.tile([48, 2 * HW], BF16)      # exp(hs^T): (u, h*HW + i)
        ewtd = pool.tile([128, 2 * HW], BF16)    # exp(ws^T) dup'd: ((dv,v), h*HW + i)

        # ---- phase A: projections + logits + exp + den ----
        with tc.tile_pool(name="psA", bufs=3, space="PSUM") as psA:
            # q for both heads
            for h in range(2):
                for b in range(NBLK):
                    qp = psA.tile([128, 512], F32, tag="pa")
                    nc.tensor.matmul(qp[:], wqt[:, h * 128:(h + 1) * 128],
                                     fmapb[:, b * 512:(b + 1) * 512],
                                     start=True, stop=True)
                    nc.scalar.copy(q2[:, h * HW + b * 512: h * HW + (b + 1) * 512], qp[:])

            # V in (j, d) layout, both heads
            for c in range(NCHUNK):
                vp = psA.tile([128, 256], F32, tag="pa")
                nc.tensor.matmul(vp[:], fmapb[:, c * 128:(c + 1) * 128], wvt[:],
                                 start=True, stop=True)
                nc.vector.tensor_copy(v2[:, c * 256:(c + 1) * 256], vp[:])

        q2v = q2[:, :].rearrange("p (h x y) -> p h x y", h=2, x=H, y=W)

        with tc.tile_pool(name="psL", bufs=3, space="PSUM") as psL:
            # hs^T -> exp -> eht ; groups of 4 x per psum bank
            for xg in range(H // 4):
                hsp = psL.tile([48, 512], F32, tag="pl")
                for xi in range(4):
                    x = xg * 4 + xi
                    nc.tensor.matmul(hsp[:, xi * 128:(xi + 1) * 128],
                                     het[:, x * 48:(x + 1) * 48],
                                     q2v[:, :, x, :], start=True, stop=True)
                src = hsp[:, :].rearrange("p (x h y) -> p x h y", x=4, h=2, y=W)
                dsl = eht[:, xg * 4 * W: xg * 4 * W + 1]
                dst = AP(dsl.tensor, dsl.offset, [dsl.ap[0], [W, 4], [HW, 2], [1, W]])
                nc.scalar.activation(dst, src, mybir.ActivationFunctionType.Exp)

            # ws^T -> exp -> ewtd[0:64] ; groups of 4 y per psum bank
            for yg in range(W // 4):
                wsp = psL.tile([64, 512], F32, tag="pl")
                for yi in range(4):
                    y = yg * 4 + yi
                    nc.tensor.matmul(wsp[:, yi * 96: yi * 96 + 96],
                                     wet[:, y * 64:(y + 1) * 64],
                                     q2v[:, :, :, y], start=True, stop=True)
                ssl = wsp[:, :]
                src = AP(ssl.tensor, ssl.offset, [ssl.ap[0], [96, 4], [48, 2], [1, 48]])
                dsl = ewtd[0:64, yg * 4: yg * 4 + 1]
                dst = AP(dsl.tensor, dsl.offset, [dsl.ap[0], [1, 4], [HW, 2], [W, 48]])
                nc.scalar.activation(dst, src, mybir.ActivationFunctionType.Exp)

        # duplicate Ewt into partitions 64..127
        nc.sync.dma_start(ewtd[64:128, :], ewtd[0:64, :])

        # denominators: den[2h] = sum_u eht, den[2h+1] = sum_v ewt
        with tc.tile_pool(name="psD", bufs=1, space="PSUM") as psD:
            for h in range(2):
                for kind in range(2):
                    dp = psD.tile([1, HW], F32, tag="pd", name=f"dp{h}{kind}")
                    for b in range(NBLK):
                        if kind == 0:
                            nc.tensor.matmul(dp[:, b * 512:(b + 1) * 512], ones48[:],
                                             eht[:, h * HW + b * 512: h * HW + (b + 1) * 512],
                                             start=True, stop=True)
                        else:
                            nc.tensor.matmul(dp[:, b * 512:(b + 1) * 512], ones64[:],
                                             ewtd[0:64, h * HW + b * 512: h * HW + (b + 1) * 512],
                                             start=True, stop=True)
                    dsb = pool.tile([1, HW], F32, name=f"densb{h}{kind}")
                    nc.scalar.copy(dsb[:], dp[:])
                    nc.sync.dma_start(den_d[2 * h + kind: 2 * h + kind + 1, :], dsb[:])

        # ---- phase B: E^T chunks, attn@V, projection ----
        with tc.tile_pool(name="eb", bufs=3) as ebpool, \
             tc.tile_pool(name="et", bufs=3) as etpool, \
             tc.tile_pool(name="nm", bufs=2) as nmpool, \
             tc.tile_pool(name="psO", bufs=6, space="PSUM") as psO, \
             tc.tile_pool(name="psP", bufs=2, space="PSUM") as psP:
            for h in range(2):
                outp = [psO.tile([128, 512], F32, tag="po", name=f"outp_h{h}_{b}")
                        for b in range(NBLK)]
                for c in range(NCHUNK):
                    ehtb = ebpool.tile([128, HW], BF16, tag="eb")
                    for du in range(2):
                        srcrow = eht[2 * c + du: 2 * c + du + 1, h * HW:(h + 1) * HW]
                        if (h, c) in GPS_CHUNKS:
                            nc.gpsimd.partition_broadcast(
                                ehtb[du * 64:(du + 1) * 64, :], srcrow)
                        else:
                            bsrc = AP(srcrow.tensor, srcrow.offset,
                                      [srcrow.ap[0], [0, 64], [1, HW]])
                            nc.sync.dma_start(ehtb[du * 64:(du + 1) * 64, :], bsrc)
                    et = etpool.tile([128, HW], BF16, tag="et")
                    nc.vector.tensor_mul(et[:], ewtd[:, h * HW:(h + 1) * HW], ehtb[:])
                    for b in range(NBLK):
                        nc.tensor.matmul(outp[b][:],
                                         v2[:, c * 256 + h * 128: c * 256 + (h + 1) * 128],
                                         et[:, b * 512:(b + 1) * 512],
                                         start=(c == 0), stop=(c == NCHUNK - 1))
                # numerator -> sbuf bf16
                numh = nmpool.tile([128, HW], BF16, tag="nm")
                for b in range(NBLK):
                    nc.scalar.copy(numh[:, b * 512:(b + 1) * 512], outp[b][:])
                # projection, gamma-scaled; psum-block -> sbuf -> DRAM
                for b in range(NBLK):
                    pp = psP.tile([128, 512], F32, tag="pp")
                    nc.tensor.matmul(pp[:], wpt[:, h * 128:(h + 1) * 128],
                                     numh[:, b * 512:(b + 1) * 512],
                                     start=True, stop=True)
                    po = nmpool.tile([128, 512], F32, tag="po")
                    nc.scalar.copy(po[:], pp[:])
                    nc.sync.dma_start(out_d[h][:, b * 512:(b + 1) * 512], po[:])

    nc.compile()
    _cached["nc"] = nc
    return nc


def _prep_core_inputs(fmap_cb, Wqk, Wv, rel_h, rel_w, Wp, gamma, pair):
    """Host-side input prep for one core. fmap_cb: (128, HW) f32 slice."""
    bf = ml_dtypes.bfloat16
    hg0 = pair * 2  # global head index of local head 0
    wqt = np.empty((128, 256), np.float32)
    wvt = np.empty((128, 256), np.float32)
    wpt = np.empty((128, 256), np.float32)
    for hl in range(2):
        hg = hg0 + hl
        wqt[:, hl * 128:(hl + 1) * 128] = SCALE * Wqk[hg * 128:(hg + 1) * 128, :].T
        wvt[:, hl * 128:(hl + 1) * 128] = Wv[hg * 128:(hg + 1) * 128, :].T
        # wpt[d, hl*128 + c] = gamma * Wp[c, hg*128 + d]
        wpt[:, hl * 128:(hl + 1) * 128] = gamma * Wp[:, hg * 128:(hg + 1) * 128].T
    idx_h = np.arange(H)[:, None] - np.arange(H)[None, :] + (MAX_POS - 1)
    idx_w = np.arange(W)[:, None] - np.arange(W)[None, :] + (MAX_POS - 1)
    het = rel_h[idx_h].transpose(2, 0, 1).reshape(128, H * H)  # (d, x*48+u)
    wet = rel_w[idx_w].transpose(2, 0, 1).reshape(128, W * W)  # (d, y*64+v)
    return {
        "fmapb": fmap_cb.astype(bf),
        "wqt": wqt.astype(bf),
        "wvt": wvt.astype(bf),
        "wpt": wpt.astype(bf),
        "het": het.astype(bf),
        "wet": wet.astype(bf),
    }


def kernel(fmap1, fmap2, Wqk, Wv, rel_h, rel_w, Wp, gamma):
    fmap1 = np.asarray(fmap1, np.float32)
    fmap2 = np.asarray(fmap2, np.float32)
    Wqk = np.asarray(Wqk, np.float32)
    Wv = np.asarray(Wv, np.float32)
    rel_h = np.asarray(rel_h, np.float32)
    rel_w = np.asarray(rel_w, np.float32)
    Wp = np.asarray(Wp, np.float32)
    g = float(np.asarray(gamma).reshape(-1)[0])

    nc = _build_nc()
    fmaps = [fmap1, fmap2]
    in_maps = []
    core_meta = []
    for pair in range(2):
        for f in range(2):
            for b in range(B):
                fm = fmaps[f][b].reshape(DIM, HW)
                in_maps.append(_prep_core_inputs(fm, Wqk, Wv, rel_h, rel_w, Wp, g, pair))
                core_meta.append((pair, f, b))

    res = bass_utils.run_bass_kernel_spmd(nc, in_maps, core_ids=list(range(8)))

    outs = [np.array(fmaps[f], np.float32).copy() for f in range(2)]
    for core, (pair, f, b) in enumerate(core_meta):
        r = res.results[core]
        den = r["den"]
        for hl in range(2):
            num = r[f"out{hl}"]                       # (128, HW) gamma-scaled numerator
            d = den[2 * hl] * den[2 * hl + 1]          # (HW,)
            outs[f][b] += (num / d[None, :]).reshape(DIM, H, W)
    return outs[0], outs[1]
